# revision 19
# baseline (speedup 1.0000x reference)
"""Causal self-attention (RoPE-3D + QK-RMSNorm) on 8 TRN2 NeuronCores.

Tensor-parallel over heads: 2 heads per core. Host shards W_qkv rows /
W_out columns, replicates x (pre-transposed), precomputes fused RoPE
cos/sin tables, and sums the 8 per-core partial projection outputs.

Per-core device kernel (Bass/Tile, no collectives):
  Phase A (per 512-token block): QKV projection (fp32r matmuls),
    QK-RMSNorm via ones-matmul partition reduction + Rsqrt +
    partition_broadcast, RoPE via fused tables + stream_shuffle.
    q,k feature-major [96, tokens]; v token-major [tokens, 96(+1 ones col)].
  Phase B (per 512 q-token i-block): S^T = k_tile^T q_block (fp32r),
    exp on ACT (no max subtraction needed: |S| <= sqrt(D)), causal mask via
    affine_select, AV + softmax-denominator via [v|1] matmul (bf16),
    normalize with Reciprocal + partition_broadcast, out-projection (bf16)
    into a [C, tokens] partial that the host sums across cores.
"""

import math
from contextlib import ExitStack

import numpy as np
import ml_dtypes

import concourse.bass as bass
import concourse.mybir as mybir
import concourse.tile as tile
from concourse import bacc
from concourse.bass_utils import run_bass_kernel_spmd

B, T, C = 2, 2048, 1536
H, D = 16, 96
NT = B * T                    # 4096 tokens
NCORES = 8
HPC = H // NCORES             # heads per core
ROPE_BASE = 10000.0

F32 = mybir.dt.float32
F32R = mybir.dt.float32r
BF16 = mybir.dt.bfloat16

KT = C // 128                 # 12 contraction tiles over C
NBLK = NT // 512              # 8 token blocks
IB_PER_B = T // 512           # 4 q i-blocks per batch
VSTRIDE = 32 * 97             # v_sb per-head columns: 32 token-tiles x (96+1)

_CACHE = {}


# ----------------------------------------------------------------- host side

def _host_tables(coords, token_type, q_scale, k_scale):
    tt = (np.asarray(token_type).reshape(NT) > 0)
    half = 16
    inv_freq = ROPE_BASE ** (-np.arange(half, dtype=np.float64) / half)
    cf = np.empty((NT, D), np.float64)
    sf = np.empty((NT, D), np.float64)
    cflat = np.asarray(coords).reshape(NT, 3).astype(np.float64)
    for a in range(3):
        ang = cflat[:, a:a + 1] * inv_freq[None, :]
        c, s = np.cos(ang), np.sin(ang)
        cf[:, a * 32:a * 32 + 16] = c
        cf[:, a * 32 + 16:a * 32 + 32] = c
        sf[:, a * 32:a * 32 + 16] = -s
        sf[:, a * 32 + 16:a * 32 + 32] = s
    cf[~tt] = 1.0
    sf[~tt] = 0.0
    pi = (np.arange(D) // 32) * 32 + (np.arange(D) + 16) % 32
    c0 = 1.0 / math.sqrt(D)
    q_scale = np.asarray(q_scale, np.float64)
    k_scale = np.asarray(k_scale, np.float64)
    cosq = np.ascontiguousarray((cf * (q_scale[None, :] * c0)).T).astype(np.float32)
    sinq = np.ascontiguousarray((sf * (q_scale[pi][None, :] * c0)).T).astype(np.float32)
    cosk = np.ascontiguousarray((cf * k_scale[None, :]).T).astype(np.float32)
    sink = np.ascontiguousarray((sf * k_scale[pi][None, :]).T).astype(np.float32)
    return cosq, sinq, cosk, sink


def _make_in_maps(x, coords, token_type, W_qkv, W_out, q_scale, k_scale):
    x = np.asarray(x, np.float32)
    W_qkv = np.asarray(W_qkv, np.float32)
    W_out = np.asarray(W_out, np.float32)
    xT = np.ascontiguousarray(x.reshape(NT, C).T)
    xbT = xT.astype(ml_dtypes.bfloat16)
    cosq, sinq, cosk, sink = _host_tables(coords, token_type, q_scale, k_scale)
    in_maps = []
    for ci in range(NCORES):
        h0 = HPC * ci
        rows = np.concatenate([
            W_qkv[h0 * D:(h0 + HPC) * D],
            W_qkv[C + h0 * D:C + (h0 + HPC) * D],
        ], axis=0)                                        # [384, C] q,k rows
        wqkvT = np.ascontiguousarray(rows.T)              # [C, 384]
        wvT = np.ascontiguousarray(
            W_qkv[2 * C + h0 * D:2 * C + (h0 + HPC) * D].T
        ).astype(ml_dtypes.bfloat16)                      # [C, 192] bf16
        woT = np.ascontiguousarray(
            W_out[:, h0 * D:(h0 + HPC) * D].T
        ).astype(ml_dtypes.bfloat16)                      # [192, C] bf16
        in_maps.append({
            "xT": xT, "wqkvT": wqkvT, "wvT": wvT, "woT": woT,
            "onesp": np.ones((128, 1), np.float32), "xbT": xbT,
            "ones96p": np.ones((1, 96), np.float32),
            "cosq": cosq, "sinq": sinq, "cosk": cosk, "sink": sink,
        })
    return in_maps


# --------------------------------------------------------------- bass builder

SWAP16 = [(i + 16) % 32 for i in range(32)]


def _build():
    nc = bacc.Bacc("TRN2", target_bir_lowering=False, debug=False)
    AF = mybir.ActivationFunctionType

    xT = nc.declare_dram_parameter("xT", [C, NT], F32R, isOutput=False)
    xbT = nc.declare_dram_parameter("xbT", [C, NT], BF16, isOutput=False)
    wqkvT = nc.declare_dram_parameter("wqkvT", [C, 2 * HPC * D], F32R, isOutput=False)
    wvT = nc.declare_dram_parameter("wvT", [C, HPC * D], BF16, isOutput=False)
    woT = nc.declare_dram_parameter("woT", [HPC * D, C], BF16, isOutput=False)
    cosq = nc.declare_dram_parameter("cosq", [D, NT], F32, isOutput=False)
    sinq = nc.declare_dram_parameter("sinq", [D, NT], F32, isOutput=False)
    cosk = nc.declare_dram_parameter("cosk", [D, NT], F32, isOutput=False)
    sink = nc.declare_dram_parameter("sink", [D, NT], F32, isOutput=False)
    onesp = nc.declare_dram_parameter("onesp", [128, 1], F32R, isOutput=False)
    ones96p = nc.declare_dram_parameter("ones96p", [1, 96], F32R, isOutput=False)
    outT = nc.declare_dram_parameter("outT", [C, NT], BF16, isOutput=True)

    with ExitStack() as ctx:
        tc = ctx.enter_context(tile.TileContext(nc))
        resid = ctx.enter_context(tc.tile_pool(name="resid", bufs=1))
        xp = ctx.enter_context(tc.tile_pool(name="xp", bufs=13))
        xbp = ctx.enter_context(tc.tile_pool(name="xbp", bufs=13))
        tp = ctx.enter_context(tc.tile_pool(name="tp", bufs=5))
        wk = ctx.enter_context(tc.tile_pool(name="wk", bufs=4))
        rp = ctx.enter_context(tc.tile_pool(name="rp", bufs=3))
        ep = ctx.enter_context(tc.tile_pool(name="ep", bufs=4))
        op_ = ctx.enter_context(tc.tile_pool(name="op", bufs=2))
        obp = ctx.enter_context(tc.tile_pool(name="obp", bufs=2))
        psA = ctx.enter_context(tc.tile_pool(name="psA", bufs=4, space="PSUM"))
        psU = ctx.enter_context(tc.tile_pool(name="psU", bufs=1, space="PSUM"))
        psW = ctx.enter_context(tc.tile_pool(name="psW", bufs=3, space="PSUM"))

        # ---- residents
        wq_sb = resid.tile([128, KT * 384], F32R, tag="wq")
        nc.sync.dma_start(
            out=wq_sb[:].rearrange("p (a f) -> p a f", a=KT),
            in_=wqkvT[:, :].rearrange("(a p) f -> p a f", p=128),
        )
        wv_sb = resid.tile([128, KT * 192], BF16, tag="wv")
        nc.sync.dma_start(
            out=wv_sb[:].rearrange("p (a f) -> p a f", a=KT),
            in_=wvT[:, :].rearrange("(a p) f -> p a f", p=128),
        )
        wo_sb = resid.tile([96, HPC * C], BF16, tag="wo")
        nc.sync.dma_start(
            out=wo_sb[:].rearrange("p (h f) -> p h f", h=HPC),
            in_=woT[:, :].rearrange("(h p) f -> p h f", p=96),
        )
        qT_sb = resid.tile([96, HPC * NT], F32R, tag="qT")
        kT_sb = resid.tile([96, HPC * NT], F32R, tag="kT")
        v_sb = resid.tile([128, HPC * VSTRIDE], BF16, tag="v")
        ones_sb = resid.tile([128, 1], F32R, tag="ones")
        nc.sync.dma_start(out=ones_sb[:], in_=onesp[:, :])
        ones96_sb = resid.tile([1, 96], F32R, tag="ones96")
        nc.sync.dma_start(out=ones96_sb[:], in_=ones96p[:, :])
        eps_sb = resid.tile([1, 1], F32, tag="eps")
        nc.gpsimd.memset(eps_sb[:], 1e-6)
        # ones column of each v token-tile
        nc.gpsimd.memset(
            v_sb[:].rearrange("p (h t f) -> p h t f", h=HPC, t=32)[:, :, :, 96:97],
            1.0,
        )

        tabs = {"q": (cosq, sinq), "k": (cosk, sink)}

        def phase_a(n):
            """QKV + norm + rope for token block n (512 tokens)."""
            xt = []
            xb = []
            for kt in range(KT):
                t = xp.tile([128, 512], F32R, tag="xt")
                nc.sync.dma_start(
                    out=t[:], in_=xT[kt * 128:(kt + 1) * 128, n * 512:(n + 1) * 512])
                xt.append(t)
                tb = xbp.tile([128, 512], BF16, tag="xb")
                nc.sync.dma_start(
                    out=tb[:], in_=xbT[kt * 128:(kt + 1) * 128, n * 512:(n + 1) * 512])
                xb.append(tb)
            # q0,q1,k0,k1 feature-major: two passes so ACT batches its
            # Square and Abs_reciprocal_sqrt table loads.
            pas = []
            for g in range(2 * HPC):
                wcol = g * 96
                pa = psA.tile([96, 512], F32, tag="pA")
                for kt in range(KT):
                    nc.tensor.matmul(
                        pa[:],
                        lhsT=wq_sb[:, kt * 384 + wcol:kt * 384 + wcol + 96],
                        rhs=xt[kt][:],
                        start=(kt == 0), stop=(kt == KT - 1),
                    )
                sq = wk.tile([96, 512], F32R, tag="sq")
                nc.scalar.activation(sq[:], pa[:], AF.Square)
                pas.append((pa, sq))
            for g in range(2 * HPC):
                qk = "q" if g < HPC else "k"
                hh = g % HPC
                pa, sq = pas[g]
                ssq = psW.tile([1, 512], F32, tag="pW")
                nc.tensor.matmul(
                    ssq[:], lhsT=ones_sb[0:96, 0:1],
                    rhs=sq[:], start=True, stop=True,
                )
                zl = rp.tile([1, 512], F32, tag="row")
                nc.scalar.activation(zl[:], ssq[:], AF.Ln,
                                     scale=1.0 / D, bias=eps_sb[:])
                rinv = rp.tile([1, 512], F32R, tag="row")
                nc.scalar.activation(rinv[:], zl[:], AF.Exp, scale=-0.5)
                bq = psW.tile([96, 512], F32, tag="pW")
                nc.tensor.matmul(bq[:], lhsT=ones96_sb[:, :], rhs=rinv[:],
                                 start=True, stop=True)
                ctab, stab = tabs[qk]
                tco = tp.tile([96, 512], F32, tag="tbl")
                nc.sync.dma_start(out=tco[:], in_=ctab[0:96, n * 512:(n + 1) * 512])
                tsi = tp.tile([96, 512], F32, tag="tbl")
                nc.sync.dma_start(out=tsi[:], in_=stab[0:96, n * 512:(n + 1) * 512])
                m1 = wk.tile([96, 512], F32, tag="m1")
                nc.vector.tensor_mul(m1[:], pa[:], tco[:])
                qsh = wk.tile([96, 512], F32, tag="qsh")
                nc.vector.stream_shuffle(qsh[:], pa[:], mask=SWAP16)
                nc.vector.tensor_mul(qsh[:], qsh[:], tsi[:])
                nc.vector.tensor_add(m1[:], m1[:], qsh[:])
                dest = (qT_sb if qk == "q" else kT_sb)
                nc.vector.tensor_mul(
                    dest[:, hh * NT + n * 512:hh * NT + (n + 1) * 512], m1[:], bq[:])
            # v token-major (bf16)
            for m in range(4):
                pv = psW.tile([128, HPC * 96], F32, tag="pW")
                for kt in range(KT):
                    nc.tensor.matmul(
                        pv[:],
                        lhsT=xb[kt][:, m * 128:(m + 1) * 128],
                        rhs=wv_sb[:, kt * 192:(kt + 1) * 192],
                        start=(kt == 0), stop=(kt == KT - 1),
                    )
                tt = n * 4 + m
                for hh in range(HPC):
                    nc.vector.tensor_copy(
                        v_sb[:, hh * VSTRIDE + tt * 97:hh * VSTRIDE + tt * 97 + 96],
                        pv[:, hh * 96:(hh + 1) * 96])

        def phase_b_attn(b, ib):
            """Attention for q i-block ib of batch b; returns per-head ou."""
            tok0 = b * T + ib * 512
            njt = 4 * ib + 4
            ous = []
            for hh in range(HPC):
                ups = psU.tile([97, 512], F32, tag="pS", name=f"ups{hh}")
                for jt in range(njt):
                    sps = psA.tile([128, 512], F32, tag="pA")
                    jtok = b * T + jt * 128
                    nc.tensor.matmul(
                        sps[:],
                        lhsT=kT_sb[:, hh * NT + jtok:hh * NT + jtok + 128],
                        rhs=qT_sb[:, hh * NT + tok0:hh * NT + tok0 + 512],
                        start=True, stop=True,
                    )
                    es = ep.tile([128, 512], BF16, tag="es")
                    nc.scalar.activation(es[:], sps[:], AF.Exp)
                    if jt >= 4 * ib:
                        s = jt - 4 * ib
                        # keep where (q idx) i - 128*s - j >= 0
                        nc.gpsimd.affine_select(
                            out=es[:], in_=es[:],
                            compare_op=mybir.AluOpType.is_ge,
                            fill=0.0, base=-128 * s, channel_multiplier=-1,
                            pattern=[[1, 512]],
                        )
                    gt = b * 16 + jt
                    nc.tensor.matmul(
                        ups[:],
                        lhsT=v_sb[:, hh * VSTRIDE + gt * 97:hh * VSTRIDE + gt * 97 + 97],
                        rhs=es[:],
                        start=(jt == 0), stop=(jt == njt - 1),
                    )
                u_sb = wk.tile([96, 512], F32, tag="usb")
                nc.scalar.activation(u_sb[:], ups[0:96, :], AF.Copy)
                zl = rp.tile([1, 512], F32, tag="row")
                nc.scalar.activation(zl[:], ups[96:97, :], AF.Ln)
                zi = rp.tile([1, 512], F32R, tag="row")
                nc.scalar.activation(zi[:], zl[:], AF.Exp, scale=-1.0)
                bz = psW.tile([96, 512], F32, tag="pW")
                nc.tensor.matmul(bz[:], lhsT=ones96_sb[:, :], rhs=zi[:],
                                 start=True, stop=True)
                ou = op_.tile([96, 512], BF16, tag=f"ou{hh}")
                nc.vector.tensor_mul(ou[:], u_sb[:], bz[:])
                ous.append(ou)
            return ous

        def phase_b_proj(b, ib, ous):
            tok0 = b * T + ib * 512
            for ct in range(KT):
                ops = psW.tile([128, 512], F32, tag="pW")
                for hh in range(HPC):
                    nc.tensor.matmul(
                        ops[:],
                        lhsT=wo_sb[:, hh * C + ct * 128:hh * C + ct * 128 + 128],
                        rhs=ous[hh][:],
                        start=(hh == 0), stop=(hh == HPC - 1),
                    )
                ob = obp.tile([128, 512], BF16, tag="ob")
                nc.vector.tensor_copy(ob[:], ops[:])
                nc.sync.dma_start(
                    out=outT[ct * 128:(ct + 1) * 128, tok0:tok0 + 512], in_=ob[:])

        # interleave: A(n) -> proj(n-1) -> attn(n), so the out-projection of
        # the previous i-block fills PE while attn's softmax tail resolves.
        prev = None
        for n in range(NBLK):
            phase_a(n)
            if prev is not None:
                phase_b_proj(*prev)
            b, ib = divmod(n, IB_PER_B)
            prev = (b, ib, phase_b_attn(b, ib))
        phase_b_proj(*prev)

    nc.compile()
    return nc


def _get_nc():
    if "nc" not in _CACHE:
        _CACHE["nc"] = _build()
    return _CACHE["nc"]


# ------------------------------------------------------------------ entrypoint

def _run(inputs, trace=False, **kw):
    nc = _get_nc()
    in_maps = _make_in_maps(**inputs)
    res = run_bass_kernel_spmd(nc, in_maps, core_ids=list(range(NCORES)),
                               trace=trace, **kw)
    acc = np.zeros((C, NT), np.float64)
    for r in res.results:
        acc += r["outT"].astype(np.float64)
    out = np.ascontiguousarray(acc.T.astype(np.float32)).reshape(B, T, C)
    return out, res


def kernel(**inputs) -> np.ndarray:
    out, _ = _run(inputs, trace=False)
    return out


# revision 22
# speedup vs baseline: 1.0208x; 1.0208x over previous
"""Causal self-attention (RoPE-3D + QK-RMSNorm) on 8 TRN2 NeuronCores.

Tensor-parallel over heads: 2 heads per core. Host shards W_qkv rows /
W_out columns, replicates x (pre-transposed), precomputes fused RoPE
cos/sin tables, and sums the 8 per-core partial projection outputs.

Per-core device kernel (Bass/Tile, no collectives):
  Phase A (per 512-token block): QKV projection (fp32r matmuls),
    QK-RMSNorm via ones-matmul partition reduction + Rsqrt +
    partition_broadcast, RoPE via fused tables + stream_shuffle.
    q,k feature-major [96, tokens]; v token-major [tokens, 96(+1 ones col)].
  Phase B (per 512 q-token i-block): S^T = k_tile^T q_block (fp32r),
    exp on ACT (no max subtraction needed: |S| <= sqrt(D)), causal mask via
    affine_select, AV + softmax-denominator via [v|1] matmul (bf16),
    normalize with Reciprocal + partition_broadcast, out-projection (bf16)
    into a [C, tokens] partial that the host sums across cores.
"""

import math
from contextlib import ExitStack

import numpy as np
import ml_dtypes

import concourse.bass as bass
import concourse.mybir as mybir
import concourse.tile as tile
from concourse import bacc
from concourse.bass_utils import run_bass_kernel_spmd

B, T, C = 2, 2048, 1536
H, D = 16, 96
NT = B * T                    # 4096 tokens
NCORES = 8
HPC = H // NCORES             # heads per core
ROPE_BASE = 10000.0

F32 = mybir.dt.float32
F32R = mybir.dt.float32r
BF16 = mybir.dt.bfloat16

KT = C // 128                 # 12 contraction tiles over C
NBLK = NT // 512              # 8 token blocks
IB_PER_B = T // 512           # 4 q i-blocks per batch
VSTRIDE = 32 * 97             # v_sb per-head columns: 32 token-tiles x (96+1)

_CACHE = {}


# ----------------------------------------------------------------- host side

def _host_tables(coords, token_type, q_scale, k_scale):
    tt = (np.asarray(token_type).reshape(NT) > 0)
    half = 16
    inv_freq = ROPE_BASE ** (-np.arange(half, dtype=np.float64) / half)
    cf = np.empty((NT, D), np.float64)
    sf = np.empty((NT, D), np.float64)
    cflat = np.asarray(coords).reshape(NT, 3).astype(np.float64)
    for a in range(3):
        ang = cflat[:, a:a + 1] * inv_freq[None, :]
        c, s = np.cos(ang), np.sin(ang)
        cf[:, a * 32:a * 32 + 16] = c
        cf[:, a * 32 + 16:a * 32 + 32] = c
        sf[:, a * 32:a * 32 + 16] = -s
        sf[:, a * 32 + 16:a * 32 + 32] = s
    cf[~tt] = 1.0
    sf[~tt] = 0.0
    pi = (np.arange(D) // 32) * 32 + (np.arange(D) + 16) % 32
    c0 = 1.0 / math.sqrt(D)
    q_scale = np.asarray(q_scale, np.float64)
    k_scale = np.asarray(k_scale, np.float64)
    cosq = np.ascontiguousarray((cf * (q_scale[None, :] * c0)).T).astype(np.float32)
    sinq = np.ascontiguousarray((sf * (q_scale[pi][None, :] * c0)).T).astype(np.float32)
    cosk = np.ascontiguousarray((cf * k_scale[None, :]).T).astype(np.float32)
    sink = np.ascontiguousarray((sf * k_scale[pi][None, :]).T).astype(np.float32)
    return cosq, sinq, cosk, sink


def _make_in_maps(x, coords, token_type, W_qkv, W_out, q_scale, k_scale):
    x = np.asarray(x, np.float32)
    W_qkv = np.asarray(W_qkv, np.float32)
    W_out = np.asarray(W_out, np.float32)
    xT = np.ascontiguousarray(x.reshape(NT, C).T)
    xbT = xT.astype(ml_dtypes.bfloat16)
    cosq, sinq, cosk, sink = _host_tables(coords, token_type, q_scale, k_scale)
    in_maps = []
    for ci in range(NCORES):
        h0 = HPC * ci
        rows = np.concatenate([
            W_qkv[h0 * D:(h0 + HPC) * D],
            W_qkv[C + h0 * D:C + (h0 + HPC) * D],
        ], axis=0)                                        # [384, C] q,k rows
        wqkvT = np.ascontiguousarray(rows.T)              # [C, 384]
        wvT = np.ascontiguousarray(
            W_qkv[2 * C + h0 * D:2 * C + (h0 + HPC) * D].T
        ).astype(ml_dtypes.bfloat16)                      # [C, 192] bf16
        woT = np.ascontiguousarray(
            W_out[:, h0 * D:(h0 + HPC) * D].T
        ).astype(ml_dtypes.bfloat16)                      # [192, C] bf16
        in_maps.append({
            "xT": xT, "wqkvT": wqkvT, "wvT": wvT, "woT": woT,
            "onesp": np.ones((128, 1), np.float32), "xbT": xbT,
            "ones96p": np.ones((1, 96), np.float32),
            "cosq": cosq, "sinq": sinq, "cosk": cosk, "sink": sink,
        })
    return in_maps


# --------------------------------------------------------------- bass builder

SWAP16 = [(i + 16) % 32 for i in range(32)]


def _build():
    nc = bacc.Bacc("TRN2", target_bir_lowering=False, debug=False)
    AF = mybir.ActivationFunctionType

    xT = nc.declare_dram_parameter("xT", [C, NT], F32R, isOutput=False)
    xbT = nc.declare_dram_parameter("xbT", [C, NT], BF16, isOutput=False)
    wqkvT = nc.declare_dram_parameter("wqkvT", [C, 2 * HPC * D], F32R, isOutput=False)
    wvT = nc.declare_dram_parameter("wvT", [C, HPC * D], BF16, isOutput=False)
    woT = nc.declare_dram_parameter("woT", [HPC * D, C], BF16, isOutput=False)
    cosq = nc.declare_dram_parameter("cosq", [D, NT], F32, isOutput=False)
    sinq = nc.declare_dram_parameter("sinq", [D, NT], F32, isOutput=False)
    cosk = nc.declare_dram_parameter("cosk", [D, NT], F32, isOutput=False)
    sink = nc.declare_dram_parameter("sink", [D, NT], F32, isOutput=False)
    onesp = nc.declare_dram_parameter("onesp", [128, 1], F32R, isOutput=False)
    ones96p = nc.declare_dram_parameter("ones96p", [1, 96], F32R, isOutput=False)
    outT = nc.declare_dram_parameter("outT", [C, NT], BF16, isOutput=True)

    with ExitStack() as ctx:
        tc = ctx.enter_context(tile.TileContext(nc))
        resid = ctx.enter_context(tc.tile_pool(name="resid", bufs=1))
        xp = ctx.enter_context(tc.tile_pool(name="xp", bufs=5))
        xbp = ctx.enter_context(tc.tile_pool(name="xbp", bufs=5))
        tp = ctx.enter_context(tc.tile_pool(name="tp", bufs=5))
        wk = ctx.enter_context(tc.tile_pool(name="wk", bufs=4))
        wk2 = ctx.enter_context(tc.tile_pool(name="wk2", bufs=2))
        rp = ctx.enter_context(tc.tile_pool(name="rp", bufs=3))
        ep = ctx.enter_context(tc.tile_pool(name="ep", bufs=4))
        op_ = ctx.enter_context(tc.tile_pool(name="op", bufs=2))
        obp = ctx.enter_context(tc.tile_pool(name="obp", bufs=2))
        psA = ctx.enter_context(tc.tile_pool(name="psA", bufs=4, space="PSUM"))
        psU = ctx.enter_context(tc.tile_pool(name="psU", bufs=1, space="PSUM"))
        psW = ctx.enter_context(tc.tile_pool(name="psW", bufs=3, space="PSUM"))

        # ---- residents
        wq_sb = resid.tile([128, KT * 384], F32R, tag="wq")
        nc.sync.dma_start(
            out=wq_sb[:].rearrange("p (a f) -> p a f", a=KT),
            in_=wqkvT[:, :].rearrange("(a p) f -> p a f", p=128),
        )
        wv_sb = resid.tile([128, KT * 192], BF16, tag="wv")
        nc.sync.dma_start(
            out=wv_sb[:].rearrange("p (a f) -> p a f", a=KT),
            in_=wvT[:, :].rearrange("(a p) f -> p a f", p=128),
        )
        wo_sb = resid.tile([96, HPC * C], BF16, tag="wo")
        nc.sync.dma_start(
            out=wo_sb[:].rearrange("p (h f) -> p h f", h=HPC),
            in_=woT[:, :].rearrange("(h p) f -> p h f", p=96),
        )
        qT_sb = resid.tile([96, HPC * NT], F32R, tag="qT")
        kT_sb = resid.tile([96, HPC * NT], F32R, tag="kT")
        v_sb = resid.tile([128, HPC * VSTRIDE], BF16, tag="v")
        ones_sb = resid.tile([128, 1], F32R, tag="ones")
        nc.sync.dma_start(out=ones_sb[:], in_=onesp[:, :])
        ones96_sb = resid.tile([1, 96], F32R, tag="ones96")
        nc.sync.dma_start(out=ones96_sb[:], in_=ones96p[:, :])
        eps_sb = resid.tile([1, 1], F32, tag="eps")
        nc.gpsimd.memset(eps_sb[:], 1e-6)
        # ones column of each v token-tile
        nc.gpsimd.memset(
            v_sb[:].rearrange("p (h t f) -> p h t f", h=HPC, t=32)[:, :, :, 96:97],
            1.0,
        )

        tabs = {"q": (cosq, sinq), "k": (cosk, sink)}

        def phase_a(n):
            """QKV + norm + rope for token block n (512 tokens)."""
            xtg = []
            xbg = []
            for g3 in range(KT // 3):
                t = xp.tile([128, 3 * 512], F32R, tag="xt", name=f"xt{g3}")
                nc.sync.dma_start(
                    out=t[:].rearrange("p (a f) -> p a f", a=3),
                    in_=xT[g3 * 384:(g3 + 1) * 384,
                           n * 512:(n + 1) * 512].rearrange("(a p) f -> p a f", p=128))
                xtg.append(t)
                tb = xbp.tile([128, 3 * 512], BF16, tag="xb", name=f"xb{g3}")
                nc.sync.dma_start(
                    out=tb[:].rearrange("p (a f) -> p a f", a=3),
                    in_=xbT[g3 * 384:(g3 + 1) * 384,
                            n * 512:(n + 1) * 512].rearrange("(a p) f -> p a f", p=128))
                xbg.append(tb)
            xt = [xtg[kt // 3][:, (kt % 3) * 512:(kt % 3 + 1) * 512] for kt in range(KT)]
            xb = [xbg[kt // 3][:, (kt % 3) * 512:(kt % 3 + 1) * 512] for kt in range(KT)]
            # q0,q1,k0,k1 feature-major: two passes so ACT batches its
            # Square and Abs_reciprocal_sqrt table loads.
            pas = []
            for g in range(2 * HPC):
                wcol = g * 96
                pa = psA.tile([96, 512], F32, tag="pA")
                for kt in range(KT):
                    nc.tensor.matmul(
                        pa[:],
                        lhsT=wq_sb[:, kt * 384 + wcol:kt * 384 + wcol + 96],
                        rhs=xt[kt],
                        start=(kt == 0), stop=(kt == KT - 1),
                    )
                sq = wk.tile([96, 512], F32R, tag="sq")
                nc.scalar.activation(sq[:], pa[:], AF.Square)
                pas.append((pa, sq))
            for g in range(2 * HPC):
                qk = "q" if g < HPC else "k"
                hh = g % HPC
                pa, sq = pas[g]
                ssq = psW.tile([1, 512], F32, tag="pW")
                nc.tensor.matmul(
                    ssq[:], lhsT=ones_sb[0:96, 0:1],
                    rhs=sq[:], start=True, stop=True,
                )
                rinv = rp.tile([1, 512], F32R, tag="row")
                nc.scalar.activation(rinv[:], ssq[:], AF.Abs_reciprocal_sqrt,
                                     scale=1.0 / D, bias=eps_sb[:])
                bq = psW.tile([96, 512], F32, tag="pW")
                nc.tensor.matmul(bq[:], lhsT=ones96_sb[:, :], rhs=rinv[:],
                                 start=True, stop=True)
                ctab, stab = tabs[qk]
                tco = tp.tile([96, 512], F32, tag="tbl")
                nc.gpsimd.dma_start(out=tco[:], in_=ctab[0:96, n * 512:(n + 1) * 512])
                tsi = tp.tile([96, 512], F32, tag="tbl")
                nc.gpsimd.dma_start(out=tsi[:], in_=stab[0:96, n * 512:(n + 1) * 512])
                m1 = wk2.tile([96, 512], F32, tag="m1")
                nc.vector.tensor_mul(m1[:], pa[:], tco[:])
                qsh = wk2.tile([96, 512], F32, tag="qsh")
                nc.vector.stream_shuffle(qsh[:], pa[:], mask=SWAP16)
                nc.vector.tensor_mul(qsh[:], qsh[:], tsi[:])
                nc.vector.tensor_add(m1[:], m1[:], qsh[:])
                dest = (qT_sb if qk == "q" else kT_sb)
                nc.vector.tensor_mul(
                    dest[:, hh * NT + n * 512:hh * NT + (n + 1) * 512], m1[:], bq[:])
            # v token-major (bf16)
            for m in range(4):
                pv = psW.tile([128, HPC * 96], F32, tag="pW")
                for kt in range(KT):
                    nc.tensor.matmul(
                        pv[:],
                        lhsT=xb[kt][:, m * 128:(m + 1) * 128],
                        rhs=wv_sb[:, kt * 192:(kt + 1) * 192],
                        start=(kt == 0), stop=(kt == KT - 1),
                    )
                tt = n * 4 + m
                for hh in range(HPC):
                    nc.vector.tensor_copy(
                        v_sb[:, hh * VSTRIDE + tt * 97:hh * VSTRIDE + tt * 97 + 96],
                        pv[:, hh * 96:(hh + 1) * 96])

        def phase_b_attn(b, ib):
            """Attention for q i-block ib of batch b; returns per-head ou."""
            tok0 = b * T + ib * 512
            njt = 4 * ib + 4
            ous = []
            for hh in range(HPC):
                ups = psU.tile([97, 512], F32, tag="pS", name=f"ups{hh}")
                for jt in range(njt):
                    sps = psA.tile([128, 512], F32, tag="pA")
                    jtok = b * T + jt * 128
                    nc.tensor.matmul(
                        sps[:],
                        lhsT=kT_sb[:, hh * NT + jtok:hh * NT + jtok + 128],
                        rhs=qT_sb[:, hh * NT + tok0:hh * NT + tok0 + 512],
                        start=True, stop=True,
                    )
                    es = ep.tile([128, 512], BF16, tag="es")
                    nc.scalar.activation(es[:], sps[:], AF.Exp)
                    if jt >= 4 * ib:
                        s = jt - 4 * ib
                        # keep where (q idx) i - 128*s - j >= 0
                        nc.gpsimd.affine_select(
                            out=es[:], in_=es[:],
                            compare_op=mybir.AluOpType.is_ge,
                            fill=0.0, base=-128 * s, channel_multiplier=-1,
                            pattern=[[1, 512]],
                        )
                    gt = b * 16 + jt
                    nc.tensor.matmul(
                        ups[:],
                        lhsT=v_sb[:, hh * VSTRIDE + gt * 97:hh * VSTRIDE + gt * 97 + 97],
                        rhs=es[:],
                        start=(jt == 0), stop=(jt == njt - 1),
                    )
                u_sb = wk2.tile([96, 512], F32, tag="usb")
                nc.scalar.activation(u_sb[:], ups[0:96, :], AF.Copy)
                zi = rp.tile([1, 512], F32R, tag="row")
                with nc.allow_low_precision(reason="f32r row for PE broadcast"):
                    nc.vector.reciprocal(zi[:], ups[96:97, :])
                bz = psW.tile([96, 512], F32, tag="pW")
                nc.tensor.matmul(bz[:], lhsT=ones96_sb[:, :], rhs=zi[:],
                                 start=True, stop=True)
                ou = op_.tile([96, 512], BF16, tag=f"ou{hh}")
                nc.vector.tensor_mul(ou[:], u_sb[:], bz[:])
                ous.append(ou)
            return ous

        def phase_b_proj(b, ib, ous):
            tok0 = b * T + ib * 512
            for ct in range(KT):
                ops = psW.tile([128, 512], F32, tag="pW")
                for hh in range(HPC):
                    nc.tensor.matmul(
                        ops[:],
                        lhsT=wo_sb[:, hh * C + ct * 128:hh * C + ct * 128 + 128],
                        rhs=ous[hh][:],
                        start=(hh == 0), stop=(hh == HPC - 1),
                    )
                ob = obp.tile([128, 512], BF16, tag="ob")
                nc.vector.tensor_copy(ob[:], ops[:])
                nc.gpsimd.dma_start(
                    out=outT[ct * 128:(ct + 1) * 128, tok0:tok0 + 512], in_=ob[:])

        # interleave: A(n) -> proj(n-1) -> attn(n), so the out-projection of
        # the previous i-block fills PE while attn's softmax tail resolves.
        prev = None
        for n in range(NBLK):
            phase_a(n)
            if prev is not None:
                phase_b_proj(*prev)
            b, ib = divmod(n, IB_PER_B)
            prev = (b, ib, phase_b_attn(b, ib))
        phase_b_proj(*prev)

    nc.compile()
    return nc


def _get_nc():
    if "nc" not in _CACHE:
        _CACHE["nc"] = _build()
    return _CACHE["nc"]


# ------------------------------------------------------------------ entrypoint

def _run(inputs, trace=False, **kw):
    nc = _get_nc()
    in_maps = _make_in_maps(**inputs)
    res = run_bass_kernel_spmd(nc, in_maps, core_ids=list(range(NCORES)),
                               trace=trace, **kw)
    acc = np.zeros((C, NT), np.float64)
    for r in res.results:
        acc += r["outT"].astype(np.float64)
    out = np.ascontiguousarray(acc.T.astype(np.float32)).reshape(B, T, C)
    return out, res


def kernel(**inputs) -> np.ndarray:
    out, _ = _run(inputs, trace=False)
    return out


# revision 24
# speedup vs baseline: 1.1331x; 1.1100x over previous
"""Causal self-attention (RoPE-3D + QK-RMSNorm) on 8 TRN2 NeuronCores.

Tensor-parallel over heads: 2 heads per core. Host shards W_qkv rows /
W_out columns, replicates x (pre-transposed), precomputes fused RoPE
cos/sin tables, and sums the 8 per-core partial projection outputs.

Per-core device kernel (Bass/Tile, no collectives):
  Phase A (per 512-token block): QKV projection (fp32r matmuls),
    QK-RMSNorm via ones-matmul partition reduction + Rsqrt +
    partition_broadcast, RoPE via fused tables + stream_shuffle.
    q,k feature-major [96, tokens]; v token-major [tokens, 96(+1 ones col)].
  Phase B (per 512 q-token i-block): S^T = k_tile^T q_block (fp32r),
    exp on ACT (no max subtraction needed: |S| <= sqrt(D)), causal mask via
    affine_select, AV + softmax-denominator via [v|1] matmul (bf16),
    normalize with Reciprocal + partition_broadcast, out-projection (bf16)
    into a [C, tokens] partial that the host sums across cores.
"""

import math
from contextlib import ExitStack

import numpy as np
import ml_dtypes

import concourse.bass as bass
import concourse.mybir as mybir
import concourse.tile as tile
from concourse import bacc
from concourse.bass_utils import run_bass_kernel_spmd

B, T, C = 2, 2048, 1536
H, D = 16, 96
NT = B * T                    # 4096 tokens
NCORES = 8
HPC = H // NCORES             # heads per core
ROPE_BASE = 10000.0

F32 = mybir.dt.float32
F32R = mybir.dt.float32r
BF16 = mybir.dt.bfloat16

KT = C // 128                 # 12 contraction tiles over C
NBLK = NT // 512              # 8 token blocks
IB_PER_B = T // 512           # 4 q i-blocks per batch
VSTRIDE = 32 * 97             # v_sb per-head columns: 32 token-tiles x (96+1)

_CACHE = {}


# ----------------------------------------------------------------- host side

def _host_tables(coords, token_type, q_scale, k_scale):
    tt = (np.asarray(token_type).reshape(NT) > 0)
    half = 16
    inv_freq = ROPE_BASE ** (-np.arange(half, dtype=np.float64) / half)
    cf = np.empty((NT, D), np.float64)
    sf = np.empty((NT, D), np.float64)
    cflat = np.asarray(coords).reshape(NT, 3).astype(np.float64)
    for a in range(3):
        ang = cflat[:, a:a + 1] * inv_freq[None, :]
        c, s = np.cos(ang), np.sin(ang)
        cf[:, a * 32:a * 32 + 16] = c
        cf[:, a * 32 + 16:a * 32 + 32] = c
        sf[:, a * 32:a * 32 + 16] = -s
        sf[:, a * 32 + 16:a * 32 + 32] = s
    cf[~tt] = 1.0
    sf[~tt] = 0.0
    pi = (np.arange(D) // 32) * 32 + (np.arange(D) + 16) % 32
    c0 = 1.0 / math.sqrt(D)
    q_scale = np.asarray(q_scale, np.float64)
    k_scale = np.asarray(k_scale, np.float64)
    cosq = np.ascontiguousarray((cf * (q_scale[None, :] * c0)).T).astype(np.float32)
    sinq = np.ascontiguousarray((sf * (q_scale[pi][None, :] * c0)).T).astype(np.float32)
    cosk = np.ascontiguousarray((cf * k_scale[None, :]).T).astype(np.float32)
    sink = np.ascontiguousarray((sf * k_scale[pi][None, :]).T).astype(np.float32)
    return cosq, sinq, cosk, sink


def _make_in_maps(x, coords, token_type, W_qkv, W_out, q_scale, k_scale):
    x = np.asarray(x, np.float32)
    W_qkv = np.asarray(W_qkv, np.float32)
    W_out = np.asarray(W_out, np.float32)
    xT = np.ascontiguousarray(x.reshape(NT, C).T)
    xbT = xT.astype(ml_dtypes.bfloat16)
    cosq, sinq, cosk, sink = _host_tables(coords, token_type, q_scale, k_scale)
    in_maps = []
    for ci in range(NCORES):
        h0 = HPC * ci
        rows = np.concatenate([
            W_qkv[h0 * D:(h0 + HPC) * D],
            W_qkv[C + h0 * D:C + (h0 + HPC) * D],
        ], axis=0)                                        # [384, C] q,k rows
        wqkvT = np.ascontiguousarray(rows.T)              # [C, 384]
        wvT = np.ascontiguousarray(
            W_qkv[2 * C + h0 * D:2 * C + (h0 + HPC) * D].T
        ).astype(ml_dtypes.bfloat16)                      # [C, 192] bf16
        woT = np.ascontiguousarray(
            W_out[:, h0 * D:(h0 + HPC) * D].T
        ).astype(ml_dtypes.bfloat16)                      # [192, C] bf16
        in_maps.append({
            "xT": xT, "wqkvT": wqkvT, "wvT": wvT, "woT": woT,
            "onesp": np.ones((128, 1), np.float32), "xbT": xbT,
            "ones96p": np.ones((1, 96), np.float32),
            "cosq": cosq, "sinq": sinq, "cosk": cosk, "sink": sink,
        })
    return in_maps


# --------------------------------------------------------------- bass builder

SWAP16 = [(i + 16) % 32 for i in range(32)]


def _build():
    nc = bacc.Bacc("TRN2", target_bir_lowering=False, debug=False)
    AF = mybir.ActivationFunctionType

    xT = nc.declare_dram_parameter("xT", [C, NT], F32R, isOutput=False)
    xbT = nc.declare_dram_parameter("xbT", [C, NT], BF16, isOutput=False)
    wqkvT = nc.declare_dram_parameter("wqkvT", [C, 2 * HPC * D], F32R, isOutput=False)
    wvT = nc.declare_dram_parameter("wvT", [C, HPC * D], BF16, isOutput=False)
    woT = nc.declare_dram_parameter("woT", [HPC * D, C], BF16, isOutput=False)
    cosq = nc.declare_dram_parameter("cosq", [D, NT], F32, isOutput=False)
    sinq = nc.declare_dram_parameter("sinq", [D, NT], F32, isOutput=False)
    cosk = nc.declare_dram_parameter("cosk", [D, NT], F32, isOutput=False)
    sink = nc.declare_dram_parameter("sink", [D, NT], F32, isOutput=False)
    onesp = nc.declare_dram_parameter("onesp", [128, 1], F32R, isOutput=False)
    ones96p = nc.declare_dram_parameter("ones96p", [1, 96], F32R, isOutput=False)
    outT = nc.declare_dram_parameter("outT", [C, NT], BF16, isOutput=True)

    with ExitStack() as ctx:
        tc = ctx.enter_context(tile.TileContext(nc))
        resid = ctx.enter_context(tc.tile_pool(name="resid", bufs=1))
        xp = ctx.enter_context(tc.tile_pool(name="xp", bufs=5))
        xbp = ctx.enter_context(tc.tile_pool(name="xbp", bufs=5))
        tp = ctx.enter_context(tc.tile_pool(name="tp", bufs=5))
        wk = ctx.enter_context(tc.tile_pool(name="wk", bufs=4))
        wk2 = ctx.enter_context(tc.tile_pool(name="wk2", bufs=2))
        rp = ctx.enter_context(tc.tile_pool(name="rp", bufs=4))
        ep = ctx.enter_context(tc.tile_pool(name="ep", bufs=4))
        op_ = ctx.enter_context(tc.tile_pool(name="op", bufs=2))
        obp = ctx.enter_context(tc.tile_pool(name="obp", bufs=2))
        psA = ctx.enter_context(tc.tile_pool(name="psA", bufs=2, space="PSUM"))
        psS = ctx.enter_context(tc.tile_pool(name="psS", bufs=3, space="PSUM"))
        psU = ctx.enter_context(tc.tile_pool(name="psU", bufs=1, space="PSUM"))
        psW = ctx.enter_context(tc.tile_pool(name="psW", bufs=2, space="PSUM"))
        dp = ctx.enter_context(tc.tile_pool(name="dp", bufs=4, space="DRAM"))

        # ---- residents
        wq_sb = resid.tile([128, KT * 384], F32R, tag="wq")
        nc.sync.dma_start(
            out=wq_sb[:].rearrange("p (a f) -> p a f", a=KT),
            in_=wqkvT[:, :].rearrange("(a p) f -> p a f", p=128),
        )
        wv_sb = resid.tile([128, KT * 192], BF16, tag="wv")
        nc.sync.dma_start(
            out=wv_sb[:].rearrange("p (a f) -> p a f", a=KT),
            in_=wvT[:, :].rearrange("(a p) f -> p a f", p=128),
        )
        wo_sb = resid.tile([96, HPC * C], BF16, tag="wo")
        nc.sync.dma_start(
            out=wo_sb[:].rearrange("p (h f) -> p h f", h=HPC),
            in_=woT[:, :].rearrange("(h p) f -> p h f", p=96),
        )
        qT_sb = resid.tile([96, HPC * NT], F32R, tag="qT")
        kT_sb = resid.tile([96, HPC * NT], F32R, tag="kT")
        v_sb = resid.tile([128, HPC * VSTRIDE], BF16, tag="v")
        rinvK_sb = resid.tile([128, HPC * 32], F32, tag="rinvK")
        ones_sb = resid.tile([128, 1], F32R, tag="ones")
        nc.sync.dma_start(out=ones_sb[:], in_=onesp[:, :])
        ones96_sb = resid.tile([1, 96], F32R, tag="ones96")
        nc.sync.dma_start(out=ones96_sb[:], in_=ones96p[:, :])
        eps_sb = resid.tile([1, 1], F32, tag="eps")
        nc.gpsimd.memset(eps_sb[:], 1e-6)
        # ones column of each v token-tile
        nc.gpsimd.memset(
            v_sb[:].rearrange("p (h t f) -> p h t f", h=HPC, t=32)[:, :, :, 96:97],
            1.0,
        )

        tabs = {"q": (cosq, sinq), "k": (cosk, sink)}

        def a_units(n):
            """Emission units for QKV+norm+rope of token block n."""
            units = []
            xtg, xbg = [], []
            sqs, pas, rinvs = {}, {}, {}

            def u_dma():
                for g3 in range(KT // 3):
                    t = xp.tile([128, 3 * 512], F32R, tag="xt", name=f"xt{g3}")
                    nc.sync.dma_start(
                        out=t[:].rearrange("p (a f) -> p a f", a=3),
                        in_=xT[g3 * 384:(g3 + 1) * 384,
                               n * 512:(n + 1) * 512].rearrange(
                                   "(a p) f -> p a f", p=128))
                    xtg.append(t)
                    tb = xbp.tile([128, 3 * 512], BF16, tag="xb", name=f"xb{g3}")
                    nc.sync.dma_start(
                        out=tb[:].rearrange("p (a f) -> p a f", a=3),
                        in_=xbT[g3 * 384:(g3 + 1) * 384,
                                n * 512:(n + 1) * 512].rearrange(
                                    "(a p) f -> p a f", p=128))
                    xbg.append(tb)
            units.append(u_dma)

            def mk_qk(g):
                def u():
                    pa = psA.tile([96, 512], F32, tag="pA", name=f"pa{g}")
                    for kt in range(KT):
                        nc.tensor.matmul(
                            pa[:],
                            lhsT=wq_sb[:, kt * 384 + g * 96:kt * 384 + g * 96 + 96],
                            rhs=xtg[kt // 3][:, (kt % 3) * 512:(kt % 3 + 1) * 512],
                            start=(kt == 0), stop=(kt == KT - 1),
                        )
                    sq = wk.tile([96, 512], F32R, tag="sq", name=f"sq{g}")
                    nc.scalar.activation(sq[:], pa[:], AF.Square)
                    pas[g] = pa
                    sqs[g] = sq
                return u
            units += [mk_qk(g) for g in range(2 * HPC)]

            def mk_v(m):
                def u():
                    pv = psW.tile([128, HPC * 96], F32, tag="pW", name=f"pv{m}")
                    for kt in range(KT):
                        nc.tensor.matmul(
                            pv[:],
                            lhsT=xbg[kt // 3][:, (kt % 3) * 512 + m * 128:
                                              (kt % 3) * 512 + (m + 1) * 128],
                            rhs=wv_sb[:, kt * 192:(kt + 1) * 192],
                            start=(kt == 0), stop=(kt == KT - 1),
                        )
                    tt = n * 4 + m
                    for hh in range(HPC):
                        nc.vector.tensor_copy(
                            v_sb[:, hh * VSTRIDE + tt * 97:hh * VSTRIDE + tt * 97 + 96],
                            pv[:, hh * 96:(hh + 1) * 96])
                return u
            units += [mk_v(m) for m in range(4)]

            def u_norm():
                # 4 ssq matmuls then 4 adjacent rsqrts (one table round-trip)
                ssqs = {}
                for g in range(2 * HPC):
                    ssq = psW.tile([1, 512], F32, tag="pW", name=f"ssq{g}")
                    nc.tensor.matmul(ssq[:], lhsT=ones_sb[0:96, 0:1],
                                     rhs=sqs[g][:], start=True, stop=True)
                    ssqs[g] = ssq
                for g in range(2 * HPC):
                    rinv = rp.tile([1, 512], F32R, tag="row", name=f"rinv{g}")
                    nc.scalar.activation(rinv[:], ssqs[g][:],
                                         AF.Abs_reciprocal_sqrt,
                                         scale=1.0 / D, bias=eps_sb[:])
                    rinvs[g] = rinv
                # k-side rinv rows -> column layout for the exp scale
                # (bounce through DRAM: SBUF source rows cannot scatter
                # across partitions directly)
                for hh in range(HPC):
                    scr = dp.tile([512], F32, tag="scr", name=f"scr{hh}")
                    nc.gpsimd.dma_start(out=scr[:],
                                        in_=rinvs[HPC + hh][:].bitcast(F32))
                    nc.gpsimd.dma_start(
                        out=rinvK_sb[:, hh * 32 + n * 4:hh * 32 + (n + 1) * 4],
                        in_=scr[:].rearrange("(c j) -> j c", j=128),
                    )
            units.append(u_norm)

            def mk_rope(g):
                def u():
                    qk = "q" if g < HPC else "k"
                    hh = g % HPC
                    pa = pas[g]
                    ctab, stab = tabs[qk]
                    tco = tp.tile([96, 512], F32, tag="tbl", name=f"tco{g}")
                    nc.gpsimd.dma_start(
                        out=tco[:], in_=ctab[0:96, n * 512:(n + 1) * 512])
                    tsi = tp.tile([96, 512], F32, tag="tbl", name=f"tsi{g}")
                    nc.gpsimd.dma_start(
                        out=tsi[:], in_=stab[0:96, n * 512:(n + 1) * 512])
                    m1 = wk2.tile([96, 512], F32, tag="m1")
                    nc.vector.tensor_mul(m1[:], pa[:], tco[:])
                    qsh = wk2.tile([96, 512], F32, tag="qsh")
                    nc.vector.stream_shuffle(qsh[:], pa[:], mask=SWAP16)
                    nc.vector.tensor_mul(qsh[:], qsh[:], tsi[:])
                    dest = (qT_sb if qk == "q" else kT_sb)
                    dslice = dest[:, hh * NT + n * 512:hh * NT + (n + 1) * 512]
                    if qk == "k":
                        # k-norm rides the exp scale; write rope result directly
                        nc.vector.tensor_add(dslice, m1[:], qsh[:])
                    else:
                        nc.vector.tensor_add(m1[:], m1[:], qsh[:])
                        bq = psW.tile([96, 512], F32, tag="pW", name=f"bq{g}")
                        nc.tensor.matmul(bq[:], lhsT=ones96_sb[:, :],
                                         rhs=rinvs[g][:], start=True, stop=True)
                        nc.vector.tensor_mul(dslice, m1[:], bq[:])
                return u
            units += [mk_rope(g) for g in range(2 * HPC)]
            return units

        def attn_units(b, ib, ous_out):
            """Attention units for q i-block ib of batch b; ou tiles appended
            to ous_out by the per-head tail units."""
            tok0 = b * T + ib * 512
            njt = 4 * ib + 4
            units = []
            state = {}

            def mk_j(hh, jt):
                def u():
                    if jt == 0:
                        state[hh] = psU.tile([97, 512], F32, tag="pS",
                                             name=f"ups{hh}")
                    ups = state[hh]
                    sps = psS.tile([128, 512], F32, tag="pA2")
                    jtok = b * T + jt * 128
                    nc.tensor.matmul(
                        sps[:],
                        lhsT=kT_sb[:, hh * NT + jtok:hh * NT + jtok + 128],
                        rhs=qT_sb[:, hh * NT + tok0:hh * NT + tok0 + 512],
                        start=True, stop=True,
                    )
                    es = ep.tile([128, 512], BF16, tag="es")
                    gt = b * 16 + jt
                    nc.scalar.activation(es[:], sps[:], AF.Exp,
                                         scale=rinvK_sb[:, hh * 32 + gt:
                                                        hh * 32 + gt + 1])
                    if jt >= 4 * ib:
                        s = jt - 4 * ib
                        # keep where (q idx) i - 128*s - j >= 0
                        nc.gpsimd.affine_select(
                            out=es[:], in_=es[:],
                            compare_op=mybir.AluOpType.is_ge,
                            fill=0.0, base=-128 * s, channel_multiplier=-1,
                            pattern=[[1, 512]],
                        )
                    nc.tensor.matmul(
                        ups[:],
                        lhsT=v_sb[:, hh * VSTRIDE + gt * 97:
                                  hh * VSTRIDE + gt * 97 + 97],
                        rhs=es[:],
                        start=(jt == 0), stop=(jt == njt - 1),
                    )
                return u

            def mk_tail(hh):
                def u():
                    ups = state[hh]
                    u_sb = wk2.tile([96, 512], F32, tag="usb")
                    nc.scalar.activation(u_sb[:], ups[0:96, :], AF.Copy)
                    zi = rp.tile([1, 512], F32R, tag="row", name=f"zi{hh}")
                    with nc.allow_low_precision(reason="f32r row for PE bcast"):
                        nc.vector.reciprocal(zi[:], ups[96:97, :])
                    bz = psW.tile([96, 512], F32, tag="pW", name=f"bz{hh}")
                    nc.tensor.matmul(bz[:], lhsT=ones96_sb[:, :], rhs=zi[:],
                                     start=True, stop=True)
                    ou = op_.tile([96, 512], BF16, tag=f"ou{hh}")
                    nc.vector.tensor_mul(ou[:], u_sb[:], bz[:])
                    ous_out.append(ou)
                return u

            for hh in range(HPC):
                units += [mk_j(hh, jt) for jt in range(njt)]
                units.append(mk_tail(hh))
            return units

        def proj_units(b, ib, ous):
            tok0 = b * T + ib * 512
            units = []

            def mk_ct(ct):
                def u():
                    ops = psW.tile([128, 512], F32, tag="pW", name=f"ops{ct}")
                    for hh in range(HPC):
                        nc.tensor.matmul(
                            ops[:],
                            lhsT=wo_sb[:, hh * C + ct * 128:hh * C + ct * 128 + 128],
                            rhs=ous[hh][:],
                            start=(hh == 0), stop=(hh == HPC - 1),
                        )
                    ob = obp.tile([128, 512], BF16, tag="ob")
                    nc.vector.tensor_copy(ob[:], ops[:])
                    nc.gpsimd.dma_start(
                        out=outT[ct * 128:(ct + 1) * 128, tok0:tok0 + 512],
                        in_=ob[:])
                return u
            return [mk_ct(ct) for ct in range(KT)]

        def weave(prim, sec):
            ia, ib_ = 0, 0
            la, lb = len(prim), len(sec)
            while ia < la or ib_ < lb:
                if ib_ >= lb or (ia < la and ia * lb <= ib_ * la):
                    prim[ia]()
                    ia += 1
                else:
                    sec[ib_]()
                    ib_ += 1

        # schedule: stream n = weave(A(n) + proj(n-2), attn(n-1))
        ous_hist = {}
        for n in range(NBLK):
            prim = a_units(n)
            if n >= 2:
                pb, pib = divmod(n - 2, IB_PER_B)
                prim += proj_units(pb, pib, ous_hist.pop(n - 2))
            sec = []
            if n >= 1:
                ab, aib = divmod(n - 1, IB_PER_B)
                ous_hist[n - 1] = []
                sec = attn_units(ab, aib, ous_hist[n - 1])
            weave(prim, sec)
        # tail: attn(7) woven with proj(6); then proj(7)
        ous_hist[NBLK - 1] = []
        sec = attn_units(*divmod(NBLK - 1, IB_PER_B), ous_hist[NBLK - 1])
        pb, pib = divmod(NBLK - 2, IB_PER_B)
        weave(proj_units(pb, pib, ous_hist.pop(NBLK - 2)), sec)
        for u in proj_units(*divmod(NBLK - 1, IB_PER_B), ous_hist.pop(NBLK - 1)):
            u()

    nc.compile()
    return nc


def _get_nc():
    if "nc" not in _CACHE:
        _CACHE["nc"] = _build()
    return _CACHE["nc"]


# ------------------------------------------------------------------ entrypoint

def _run(inputs, trace=False, **kw):
    nc = _get_nc()
    in_maps = _make_in_maps(**inputs)
    res = run_bass_kernel_spmd(nc, in_maps, core_ids=list(range(NCORES)),
                               trace=trace, **kw)
    acc = np.zeros((C, NT), np.float64)
    for r in res.results:
        acc += r["outT"].astype(np.float64)
    out = np.ascontiguousarray(acc.T.astype(np.float32)).reshape(B, T, C)
    return out, res


def kernel(**inputs) -> np.ndarray:
    out, _ = _run(inputs, trace=False)
    return out


# revision 26
# speedup vs baseline: 1.2125x; 1.0700x over previous
"""Causal self-attention (RoPE-3D + QK-RMSNorm) on 8 TRN2 NeuronCores.

Tensor-parallel over heads: 2 heads per core. Host shards W_qkv rows /
W_out columns, replicates x (pre-transposed), precomputes fused RoPE
cos/sin tables, and sums the 8 per-core partial projection outputs.

Per-core device kernel (Bass/Tile, no collectives):
  Phase A (per 512-token block): QKV projection (fp32r matmuls),
    QK-RMSNorm via ones-matmul partition reduction + Rsqrt +
    partition_broadcast, RoPE via fused tables + stream_shuffle.
    q,k feature-major [96, tokens]; v token-major [tokens, 96(+1 ones col)].
  Phase B (per 512 q-token i-block): S^T = k_tile^T q_block (fp32r),
    exp on ACT (no max subtraction needed: |S| <= sqrt(D)), causal mask via
    affine_select, AV + softmax-denominator via [v|1] matmul (bf16),
    normalize with Reciprocal + partition_broadcast, out-projection (bf16)
    into a [C, tokens] partial that the host sums across cores.
"""

import math
from contextlib import ExitStack

import numpy as np
import ml_dtypes

import concourse.bass as bass
import concourse.mybir as mybir
import concourse.tile as tile
from concourse import bacc
from concourse.bass_utils import run_bass_kernel_spmd

B, T, C = 2, 2048, 1536
H, D = 16, 96
NT = B * T                    # 4096 tokens
NCORES = 8
HPC = H // NCORES             # heads per core
ROPE_BASE = 10000.0

F32 = mybir.dt.float32
F32R = mybir.dt.float32r
BF16 = mybir.dt.bfloat16

KT = C // 128                 # 12 contraction tiles over C
NBLK = NT // 512              # 8 token blocks
IB_PER_B = T // 512           # 4 q i-blocks per batch
VSTRIDE = 32 * 97             # v_sb per-head columns: 32 token-tiles x (96+1)

_CACHE = {}


# ----------------------------------------------------------------- host side

def _host_tables(coords, token_type, q_scale, k_scale):
    tt = (np.asarray(token_type).reshape(NT) > 0)
    half = 16
    inv_freq = ROPE_BASE ** (-np.arange(half, dtype=np.float64) / half)
    cf = np.empty((NT, D), np.float64)
    sf = np.empty((NT, D), np.float64)
    cflat = np.asarray(coords).reshape(NT, 3).astype(np.float64)
    for a in range(3):
        ang = cflat[:, a:a + 1] * inv_freq[None, :]
        c, s = np.cos(ang), np.sin(ang)
        cf[:, a * 32:a * 32 + 16] = c
        cf[:, a * 32 + 16:a * 32 + 32] = c
        sf[:, a * 32:a * 32 + 16] = -s
        sf[:, a * 32 + 16:a * 32 + 32] = s
    cf[~tt] = 1.0
    sf[~tt] = 0.0
    pi = (np.arange(D) // 32) * 32 + (np.arange(D) + 16) % 32
    c0 = 1.0 / math.sqrt(D)
    q_scale = np.asarray(q_scale, np.float64)
    k_scale = np.asarray(k_scale, np.float64)
    cosq = np.ascontiguousarray((cf * (q_scale[None, :] * c0)).T).astype(np.float32)
    sinq = np.ascontiguousarray((sf * (q_scale[pi][None, :] * c0)).T).astype(np.float32)
    cosk = np.ascontiguousarray((cf * k_scale[None, :]).T).astype(np.float32)
    sink = np.ascontiguousarray((sf * k_scale[pi][None, :]).T).astype(np.float32)
    return cosq, sinq, cosk, sink


def _make_in_maps(x, coords, token_type, W_qkv, W_out, q_scale, k_scale):
    x = np.asarray(x, np.float32)
    W_qkv = np.asarray(W_qkv, np.float32)
    W_out = np.asarray(W_out, np.float32)
    xT = np.ascontiguousarray(x.reshape(NT, C).T)
    xbT = xT.astype(ml_dtypes.bfloat16)
    cosq, sinq, cosk, sink = _host_tables(coords, token_type, q_scale, k_scale)
    in_maps = []
    for ci in range(NCORES):
        h0 = HPC * ci
        rows = np.concatenate([
            W_qkv[h0 * D:(h0 + HPC) * D],
            W_qkv[C + h0 * D:C + (h0 + HPC) * D],
        ], axis=0)                                        # [384, C] q,k rows
        wqkvT = np.ascontiguousarray(rows.T)              # [C, 384]
        wvT = np.ascontiguousarray(
            W_qkv[2 * C + h0 * D:2 * C + (h0 + HPC) * D].T
        ).astype(ml_dtypes.bfloat16)                      # [C, 192] bf16
        woT = np.ascontiguousarray(
            W_out[:, h0 * D:(h0 + HPC) * D].T
        ).astype(ml_dtypes.bfloat16)                      # [192, C] bf16
        in_maps.append({
            "xT": xT, "wqkvT": wqkvT, "wvT": wvT, "woT": woT,
            "onesp": np.ones((128, 1), np.float32), "xbT": xbT,
            "ones96p": np.ones((1, 96), np.float32),
            "cosq": cosq, "sinq": sinq, "cosk": cosk, "sink": sink,
        })
    return in_maps


# --------------------------------------------------------------- bass builder

SWAP16 = [(i + 16) % 32 for i in range(32)]


def _build():
    nc = bacc.Bacc("TRN2", target_bir_lowering=False, debug=False)
    AF = mybir.ActivationFunctionType

    xT = nc.declare_dram_parameter("xT", [C, NT], F32R, isOutput=False)
    xbT = nc.declare_dram_parameter("xbT", [C, NT], BF16, isOutput=False)
    wqkvT = nc.declare_dram_parameter("wqkvT", [C, 2 * HPC * D], F32R, isOutput=False)
    wvT = nc.declare_dram_parameter("wvT", [C, HPC * D], BF16, isOutput=False)
    woT = nc.declare_dram_parameter("woT", [HPC * D, C], BF16, isOutput=False)
    cosq = nc.declare_dram_parameter("cosq", [D, NT], F32, isOutput=False)
    sinq = nc.declare_dram_parameter("sinq", [D, NT], F32, isOutput=False)
    cosk = nc.declare_dram_parameter("cosk", [D, NT], F32, isOutput=False)
    sink = nc.declare_dram_parameter("sink", [D, NT], F32, isOutput=False)
    onesp = nc.declare_dram_parameter("onesp", [128, 1], F32R, isOutput=False)
    ones96p = nc.declare_dram_parameter("ones96p", [1, 96], F32R, isOutput=False)
    outT = nc.declare_dram_parameter("outT", [C, NT], BF16, isOutput=True)

    with ExitStack() as ctx:
        tc = ctx.enter_context(tile.TileContext(nc))
        resid = ctx.enter_context(tc.tile_pool(name="resid", bufs=1))
        xp = ctx.enter_context(tc.tile_pool(name="xp", bufs=5))
        xbp = ctx.enter_context(tc.tile_pool(name="xbp", bufs=5))
        tp = ctx.enter_context(tc.tile_pool(name="tp", bufs=5))
        wk = ctx.enter_context(tc.tile_pool(name="wk", bufs=4))
        wk2 = ctx.enter_context(tc.tile_pool(name="wk2", bufs=2))
        rp = ctx.enter_context(tc.tile_pool(name="rp", bufs=4))
        ep = ctx.enter_context(tc.tile_pool(name="ep", bufs=4))
        op_ = ctx.enter_context(tc.tile_pool(name="op", bufs=2))
        obp = ctx.enter_context(tc.tile_pool(name="obp", bufs=2))
        psA = ctx.enter_context(tc.tile_pool(name="psA", bufs=2, space="PSUM"))
        psS = ctx.enter_context(tc.tile_pool(name="psS", bufs=3, space="PSUM"))
        psU = ctx.enter_context(tc.tile_pool(name="psU", bufs=1, space="PSUM"))
        psW = ctx.enter_context(tc.tile_pool(name="psW", bufs=2, space="PSUM"))
        dp = ctx.enter_context(tc.tile_pool(name="dp", bufs=4, space="DRAM"))

        # ---- residents
        wq_sb = resid.tile([128, KT * 384], F32R, tag="wq")
        nc.sync.dma_start(
            out=wq_sb[:].rearrange("p (a f) -> p a f", a=KT),
            in_=wqkvT[:, :].rearrange("(a p) f -> p a f", p=128),
        )
        wv_sb = resid.tile([128, KT * 192], BF16, tag="wv")
        nc.sync.dma_start(
            out=wv_sb[:].rearrange("p (a f) -> p a f", a=KT),
            in_=wvT[:, :].rearrange("(a p) f -> p a f", p=128),
        )
        wo_sb = resid.tile([96, HPC * C], BF16, tag="wo")
        nc.sync.dma_start(
            out=wo_sb[:].rearrange("p (h f) -> p h f", h=HPC),
            in_=woT[:, :].rearrange("(h p) f -> p h f", p=96),
        )
        qT_sb = resid.tile([96, HPC * NT], F32R, tag="qT")
        kT_sb = resid.tile([96, HPC * NT], F32R, tag="kT")
        v_sb = resid.tile([128, HPC * VSTRIDE], BF16, tag="v")
        rinvK_sb = resid.tile([128, HPC * 32], F32, tag="rinvK")
        ones_sb = resid.tile([128, 1], F32R, tag="ones")
        nc.sync.dma_start(out=ones_sb[:], in_=onesp[:, :])
        ones96_sb = resid.tile([1, 96], F32R, tag="ones96")
        nc.sync.dma_start(out=ones96_sb[:], in_=ones96p[:, :])
        eps_sb = resid.tile([1, 1], F32, tag="eps")
        nc.gpsimd.memset(eps_sb[:], 1e-6)
        # ones column of each v token-tile
        nc.gpsimd.memset(
            v_sb[:].rearrange("p (h t f) -> p h t f", h=HPC, t=32)[:, :, :, 96:97],
            1.0,
        )

        tabs = {"q": (cosq, sinq), "k": (cosk, sink)}

        def a_units(n):
            """Emission units for QKV+norm+rope of token block n."""
            units = []
            xtg, xbg = [], []
            sqs, pas, rinvs = {}, {}, {}

            def u_dma():
                for g3 in range(KT // 3):
                    t = xp.tile([128, 3 * 512], F32R, tag="xt", name=f"xt{g3}")
                    nc.sync.dma_start(
                        out=t[:].rearrange("p (a f) -> p a f", a=3),
                        in_=xT[g3 * 384:(g3 + 1) * 384,
                               n * 512:(n + 1) * 512].rearrange(
                                   "(a p) f -> p a f", p=128))
                    xtg.append(t)
                    tb = xbp.tile([128, 3 * 512], BF16, tag="xb", name=f"xb{g3}")
                    nc.sync.dma_start(
                        out=tb[:].rearrange("p (a f) -> p a f", a=3),
                        in_=xbT[g3 * 384:(g3 + 1) * 384,
                                n * 512:(n + 1) * 512].rearrange(
                                    "(a p) f -> p a f", p=128))
                    xbg.append(tb)
            units.append(u_dma)

            def mk_qk(g):
                def u():
                    pa = psA.tile([96, 512], F32, tag="pA", name=f"pa{g}")
                    for kt in range(KT):
                        nc.tensor.matmul(
                            pa[:],
                            lhsT=wq_sb[:, kt * 384 + g * 96:kt * 384 + g * 96 + 96],
                            rhs=xtg[kt // 3][:, (kt % 3) * 512:(kt % 3 + 1) * 512],
                            start=(kt == 0), stop=(kt == KT - 1),
                        )
                    sq = wk.tile([96, 512], F32R, tag="sq", name=f"sq{g}")
                    nc.scalar.activation(sq[:], pa[:], AF.Square)
                    pas[g] = pa
                    sqs[g] = sq
                return u
            units += [mk_qk(g) for g in range(2 * HPC)]

            def mk_v(m):
                def u():
                    pv = psW.tile([128, HPC * 96], F32, tag="pW", name=f"pv{m}")
                    for kt in range(KT):
                        nc.tensor.matmul(
                            pv[:],
                            lhsT=xbg[kt // 3][:, (kt % 3) * 512 + m * 128:
                                              (kt % 3) * 512 + (m + 1) * 128],
                            rhs=wv_sb[:, kt * 192:(kt + 1) * 192],
                            start=(kt == 0), stop=(kt == KT - 1),
                        )
                    tt = n * 4 + m
                    for hh in range(HPC):
                        nc.vector.tensor_copy(
                            v_sb[:, hh * VSTRIDE + tt * 97:hh * VSTRIDE + tt * 97 + 96],
                            pv[:, hh * 96:(hh + 1) * 96])
                return u
            units += [mk_v(m) for m in range(4)]

            def u_norm():
                # 4 ssq matmuls then 4 adjacent rsqrts (one table round-trip)
                ssqs = {}
                for g in range(2 * HPC):
                    ssq = psW.tile([1, 512], F32, tag="pW", name=f"ssq{g}")
                    nc.tensor.matmul(ssq[:], lhsT=ones_sb[0:96, 0:1],
                                     rhs=sqs[g][:], start=True, stop=True)
                    ssqs[g] = ssq
                for g in range(2 * HPC):
                    rinv = rp.tile([1, 512], F32R, tag="row", name=f"rinv{g}")
                    nc.scalar.activation(rinv[:], ssqs[g][:],
                                         AF.Abs_reciprocal_sqrt,
                                         scale=1.0 / D, bias=eps_sb[:])
                    rinvs[g] = rinv
                # k-side rinv rows -> column layout for the exp scale
                # (bounce through DRAM: SBUF source rows cannot scatter
                # across partitions directly)
                for hh in range(HPC):
                    scr = dp.tile([512], F32, tag="scr", name=f"scr{hh}")
                    nc.sync.dma_start(out=scr[:],
                                        in_=rinvs[HPC + hh][:].bitcast(F32))
                    nc.gpsimd.dma_start(
                        out=rinvK_sb[:, hh * 32 + n * 4:hh * 32 + (n + 1) * 4],
                        in_=scr[:].rearrange("(c j) -> j c", j=128),
                    )
            units.append(u_norm)

            def mk_rope(g):
                def u():
                    qk = "q" if g < HPC else "k"
                    hh = g % HPC
                    pa = pas[g]
                    ctab, stab = tabs[qk]
                    tco = tp.tile([96, 512], F32, tag="tbl", name=f"tco{g}")
                    nc.sync.dma_start(
                        out=tco[:], in_=ctab[0:96, n * 512:(n + 1) * 512])
                    tsi = tp.tile([96, 512], F32, tag="tbl", name=f"tsi{g}")
                    nc.sync.dma_start(
                        out=tsi[:], in_=stab[0:96, n * 512:(n + 1) * 512])
                    m1 = wk2.tile([96, 512], F32, tag="m1")
                    nc.vector.tensor_mul(m1[:], pa[:], tco[:])
                    qsh = wk2.tile([96, 512], F32, tag="qsh")
                    nc.vector.stream_shuffle(qsh[:], pa[:], mask=SWAP16)
                    nc.vector.tensor_mul(qsh[:], qsh[:], tsi[:])
                    dest = (qT_sb if qk == "q" else kT_sb)
                    dslice = dest[:, hh * NT + n * 512:hh * NT + (n + 1) * 512]
                    if qk == "k":
                        # k-norm rides the exp scale; write rope result directly
                        nc.vector.tensor_add(dslice, m1[:], qsh[:])
                    else:
                        nc.vector.tensor_add(m1[:], m1[:], qsh[:])
                        bq = psW.tile([96, 512], F32, tag="pW", name=f"bq{g}")
                        nc.tensor.matmul(bq[:], lhsT=ones96_sb[:, :],
                                         rhs=rinvs[g][:], start=True, stop=True)
                        nc.vector.tensor_mul(dslice, m1[:], bq[:])
                return u
            units += [mk_rope(g) for g in range(2 * HPC)]
            return units

        def attn_units(b, ib, ous_out):
            """Attention units for q i-block ib of batch b; ou tiles appended
            to ous_out by the per-head tail units."""
            tok0 = b * T + ib * 512
            njt = 4 * ib + 4
            units = []
            state = {}

            def mk_j(hh, jt):
                def u():
                    if jt == 0:
                        state[hh] = psU.tile([97, 512], F32, tag="pS",
                                             name=f"ups{hh}")
                    ups = state[hh]
                    sps = psS.tile([128, 512], F32, tag="pA2")
                    jtok = b * T + jt * 128
                    nc.tensor.matmul(
                        sps[:],
                        lhsT=kT_sb[:, hh * NT + jtok:hh * NT + jtok + 128],
                        rhs=qT_sb[:, hh * NT + tok0:hh * NT + tok0 + 512],
                        start=True, stop=True,
                    )
                    es = ep.tile([128, 512], BF16, tag="es")
                    gt = b * 16 + jt
                    nc.scalar.activation(es[:], sps[:], AF.Exp,
                                         scale=rinvK_sb[:, hh * 32 + gt:
                                                        hh * 32 + gt + 1])
                    if jt >= 4 * ib:
                        s = jt - 4 * ib
                        # keep where (q idx) i - 128*s - j >= 0
                        nc.gpsimd.affine_select(
                            out=es[:], in_=es[:],
                            compare_op=mybir.AluOpType.is_ge,
                            fill=0.0, base=-128 * s, channel_multiplier=-1,
                            pattern=[[1, 512]],
                        )
                    nc.tensor.matmul(
                        ups[:],
                        lhsT=v_sb[:, hh * VSTRIDE + gt * 97:
                                  hh * VSTRIDE + gt * 97 + 97],
                        rhs=es[:],
                        start=(jt == 0), stop=(jt == njt - 1),
                    )
                return u

            def mk_tail(hh):
                def u():
                    ups = state[hh]
                    u_sb = wk2.tile([96, 512], F32, tag="usb")
                    nc.scalar.activation(u_sb[:], ups[0:96, :], AF.Copy)
                    zr = rp.tile([1, 512], F32, tag="row", name=f"zr{hh}")
                    nc.scalar.activation(zr[:], ups[96:97, :],
                                         AF.Abs_reciprocal_sqrt)
                    zi = rp.tile([1, 512], F32R, tag="row", name=f"zi{hh}")
                    nc.scalar.activation(zi[:], zr[:], AF.Square)
                    bz = psW.tile([96, 512], F32, tag="pW", name=f"bz{hh}")
                    nc.tensor.matmul(bz[:], lhsT=ones96_sb[:, :], rhs=zi[:],
                                     start=True, stop=True)
                    ou = op_.tile([96, 512], BF16, tag=f"ou{hh}")
                    nc.vector.tensor_mul(ou[:], u_sb[:], bz[:])
                    ous_out.append(ou)
                return u

            for hh in range(HPC):
                units += [mk_j(hh, jt) for jt in range(njt)]
                units.append(mk_tail(hh))
            return units

        def proj_units(b, ib, ous):
            tok0 = b * T + ib * 512
            units = []

            def mk_ct(ct):
                def u():
                    ops = psW.tile([128, 512], F32, tag="pW", name=f"ops{ct}")
                    for hh in range(HPC):
                        nc.tensor.matmul(
                            ops[:],
                            lhsT=wo_sb[:, hh * C + ct * 128:hh * C + ct * 128 + 128],
                            rhs=ous[hh][:],
                            start=(hh == 0), stop=(hh == HPC - 1),
                        )
                    ob = obp.tile([128, 512], BF16, tag="ob")
                    nc.vector.tensor_copy(ob[:], ops[:])
                    nc.gpsimd.dma_start(
                        out=outT[ct * 128:(ct + 1) * 128, tok0:tok0 + 512],
                        in_=ob[:])
                return u
            return [mk_ct(ct) for ct in range(KT)]

        def weave(prim, sec):
            ia, ib_ = 0, 0
            la, lb = len(prim), len(sec)
            while ia < la or ib_ < lb:
                if ib_ >= lb or (ia < la and ia * lb <= ib_ * la):
                    prim[ia]()
                    ia += 1
                else:
                    sec[ib_]()
                    ib_ += 1

        # schedule: stream n = weave(A(n) + proj(n-2), attn(n-1))
        ous_hist = {}
        for n in range(NBLK):
            prim = a_units(n)
            if n >= 2:
                pb, pib = divmod(n - 2, IB_PER_B)
                prim += proj_units(pb, pib, ous_hist.pop(n - 2))
            sec = []
            if n >= 1:
                ab, aib = divmod(n - 1, IB_PER_B)
                ous_hist[n - 1] = []
                sec = attn_units(ab, aib, ous_hist[n - 1])
            weave(prim, sec)
        # tail: attn(7) woven with proj(6); then proj(7)
        ous_hist[NBLK - 1] = []
        sec = attn_units(*divmod(NBLK - 1, IB_PER_B), ous_hist[NBLK - 1])
        pb, pib = divmod(NBLK - 2, IB_PER_B)
        weave(proj_units(pb, pib, ous_hist.pop(NBLK - 2)), sec)
        for u in proj_units(*divmod(NBLK - 1, IB_PER_B), ous_hist.pop(NBLK - 1)):
            u()

    nc.compile()
    return nc


def _get_nc():
    if "nc" not in _CACHE:
        _CACHE["nc"] = _build()
    return _CACHE["nc"]


# ------------------------------------------------------------------ entrypoint

def _run(inputs, trace=False, **kw):
    nc = _get_nc()
    in_maps = _make_in_maps(**inputs)
    res = run_bass_kernel_spmd(nc, in_maps, core_ids=list(range(NCORES)),
                               trace=trace, **kw)
    acc = np.zeros((C, NT), np.float64)
    for r in res.results:
        acc += r["outT"].astype(np.float64)
    out = np.ascontiguousarray(acc.T.astype(np.float32)).reshape(B, T, C)
    return out, res


def kernel(**inputs) -> np.ndarray:
    out, _ = _run(inputs, trace=False)
    return out
